# revision 9
# baseline (speedup 1.0000x reference)
"""Feature propagation (kNN interpolate, k=3) Trainium2 kernel.

Problem: for B=4 point clouds, each with N=4096 source points (xyz, feat[256])
and M=16384 query points (new_xyz), find the 3 nearest source points per query
and inverse-distance-interpolate their features.

Sharding: 8 cores = 4 clouds x 2 query halves. Each core handles 8192 queries
against its cloud's full 4096 sources -> fully local, no collectives.

Per-core algorithm (per 128-query block):
  score t[m,n] = 2 q_m . s_n - |s_n|^2 - |q_m|^2  = -d2[m,n]
  computed as ONE K=5 matmul:  lhsT rows = [qx,qy,qz, 1, -q2],
                               rhs  rows = [2sx,2sy,2sz, -s2, 1]
  (the -q2 / -s2 rows are computed on device.)
  top-3 of t per row via DVE max/max_index (native top-8 ops),
  weights = normalized 1/(sqrt(d2)+eps), features gathered from HBM with
  per-partition indirect DMA, weighted-summed on DVE.
"""

import os
import numpy as np

import concourse.bacc as bacc
import concourse.bass as bass
import concourse.mybir as mybir
import concourse.tile as tile
from concourse.bass import IndirectOffsetOnAxis
from concourse.bass_utils import run_bass_kernel_spmd

F32 = mybir.dt.float32
U32 = mybir.dt.uint32
ALU = mybir.AluOpType
AX = mybir.AxisListType

EPS = 1e-8

# full-problem constants (hardcoded per harness contract)
B_CLOUDS = 4
N_SRC = 4096
M_QUERY = 16384
C_FEAT = 256
KNN = 3
N_CORES = 8
MQ = M_QUERY // 2  # queries per core (2 cores per cloud)

# set by kernel() after each run; test.py reads it for the profile numbers
LAST_RESULTS = None


def build_program(n_src=N_SRC, mq=MQ, c_feat=C_FEAT):
    """One NeuronCore program; identical on all cores (SPMD, different data)."""
    nc = bacc.Bacc("TRN2", target_bir_lowering=False, debug=False)

    # host rows: lhsT = [qx,qy,qz, 1, 0(-q2 filled on device)]
    #            sT   = [sx,sy,sz, 0(-s2 filled on device), 1]
    lhsT_d = nc.dram_tensor("lhsT", [5, mq], F32, kind="ExternalInput")
    sT_d = nc.dram_tensor("sT", [5, n_src], F32, kind="ExternalInput")
    feat_d = nc.dram_tensor("feat", [n_src, c_feat], F32, kind="ExternalInput")
    out_d = nc.dram_tensor("out", [mq, c_feat], F32, kind="ExternalOutput")

    nblk = mq // 128
    qseg = min(512, mq)
    sseg = min(512, n_src)
    nseg_s = n_src // sseg  # rhs prep segments
    nseg_q = mq // qseg  # lhsT prep segments
    hchunk = min(1024, n_src)  # psum chunk (free dim) per matmul group
    nh = n_src // hchunk  # psum half-blocks per query block

    with tile.TileContext(nc) as tc:
        with (
            tc.tile_pool(name="persist", bufs=1) as persist,
            tc.tile_pool(name="prep_ps", bufs=2, space="PSUM") as prep_ps,
            tc.tile_pool(name="mm_ps", bufs=3, space="PSUM") as mm_ps,
            tc.tile_pool(name="t_pool", bufs=3) as t_pool,
            tc.tile_pool(name="sm", bufs=4) as sm,
            tc.tile_pool(name="g_pool", bufs=3) as g_pool,
            tc.tile_pool(name="o_pool", bufs=3) as o_pool,
        ):
            # ---------------- one-time prep ----------------
            lhsT5 = persist.tile([5, mq], F32)  # rows: qx qy qz 1 -q2
            rhs5 = persist.tile([5, n_src], F32)  # rows: 2sx 2sy 2sz -s2 1
            ones3 = persist.tile([3, 1], F32)
            qsq = persist.tile([3, mq], F32)
            ssq = persist.tile([3, n_src], F32)
            q2neg = persist.tile([1, mq], F32)
            s2neg = persist.tile([1, n_src], F32)

            nc.sync.dma_start(out=lhsT5[:], in_=lhsT_d[:, :])
            nc.sync.dma_start(out=rhs5[:], in_=sT_d[:, :])
            nc.vector.memset(ones3[:], 1.0)

            # -q2 row: ones(3)^T @ (q .* q), negated on PSUM eviction.
            # (compute engines need partition-base 0, so build the row in a
            #  base-0 tile and DMA it into partition 4/3 of the operand tile)
            nc.vector.tensor_tensor(
                out=qsq[:], in0=lhsT5[0:3, :], in1=lhsT5[0:3, :], op=ALU.mult
            )
            for j in range(nseg_q):
                ps = prep_ps.tile([1, qseg], F32, tag="prep")
                nc.tensor.matmul(
                    ps[:],
                    lhsT=ones3[:],
                    rhs=qsq[:, j * qseg : (j + 1) * qseg],
                    start=True,
                    stop=True,
                )
                nc.scalar.mul(q2neg[:, j * qseg : (j + 1) * qseg], ps[:], -1.0)
            nc.sync.dma_start(out=lhsT5[4:5, :], in_=q2neg[:])

            # -s2 row (from unscaled s), then scale s rows by 2 in place
            nc.vector.tensor_tensor(
                out=ssq[:], in0=rhs5[0:3, :], in1=rhs5[0:3, :], op=ALU.mult
            )
            for j in range(nseg_s):
                ps = prep_ps.tile([1, sseg], F32, tag="prep")
                nc.tensor.matmul(
                    ps[:],
                    lhsT=ones3[:],
                    rhs=ssq[:, j * sseg : (j + 1) * sseg],
                    start=True,
                    stop=True,
                )
                nc.scalar.mul(s2neg[:, j * sseg : (j + 1) * sseg], ps[:], -1.0)
            nc.sync.dma_start(out=rhs5[3:4, :], in_=s2neg[:])
            nc.vector.tensor_scalar_mul(rhs5[0:3, :], rhs5[0:3, :], 2.0)

            # ---------------- per-block main loop ----------------
            for b in range(nblk):
                lhsT_blk = lhsT5[:, b * 128 : (b + 1) * 128]

                t_sb = t_pool.tile([128, n_src], F32)
                for h in range(nh):
                    ps = mm_ps.tile([128, hchunk], F32)
                    for j in range(hchunk // 512):
                        nc.tensor.matmul(
                            ps[:, j * 512 : (j + 1) * 512],
                            lhsT=lhsT_blk,
                            rhs=rhs5[
                                :, h * hchunk + j * 512 : h * hchunk + (j + 1) * 512
                            ],
                            start=True,
                            stop=True,
                        )
                    nc.scalar.copy(t_sb[:, h * hchunk : (h + 1) * hchunk], ps[:])

                m8 = sm.tile([128, 8], F32, tag="m8")
                i8 = sm.tile([128, 8], U32, tag="i8")
                nc.vector.max(m8[:], t_sb[:])
                nc.vector.max_index(i8[:], m8[:], t_sb[:])

                # d2 = max(-t, 1e-12); dist = sqrt(d2); r = 1/(dist+eps)
                d2 = sm.tile([128, 3], F32, tag="d2")
                nc.vector.tensor_scalar(
                    d2[:], m8[:, 0:3], -1.0, 1e-12, op0=ALU.mult, op1=ALU.max
                )
                dist = sm.tile([128, 3], F32, tag="dist")
                nc.scalar.sqrt(dist[:], d2[:])
                nc.vector.tensor_scalar_add(dist[:], dist[:], EPS)
                r = sm.tile([128, 3], F32, tag="r")
                nc.vector.reciprocal(r[:], dist[:])
                ws = sm.tile([128, 1], F32, tag="ws")
                nc.vector.tensor_reduce(ws[:], r[:], axis=AX.X, op=ALU.add)
                nc.vector.reciprocal(ws[:], ws[:])
                w = sm.tile([128, 3], F32, tag="w")
                nc.vector.tensor_tensor(
                    w[:], r[:], ws[:].to_broadcast([128, 3]), op=ALU.mult
                )

                g = g_pool.tile([128, KNN, c_feat], F32)
                for j in range(KNN):
                    nc.gpsimd.indirect_dma_start(
                        out=g[:, j, :],
                        out_offset=None,
                        in_=feat_d[:, :],
                        in_offset=IndirectOffsetOnAxis(ap=i8[:, j : j + 1], axis=0),
                    )

                o = o_pool.tile([128, c_feat], F32)
                nc.vector.tensor_scalar(
                    o[:], g[:, 0, :], w[:, 0:1], None, op0=ALU.mult
                )
                nc.vector.scalar_tensor_tensor(
                    o[:], g[:, 1, :], w[:, 1:2], o[:], op0=ALU.mult, op1=ALU.add
                )
                nc.vector.scalar_tensor_tensor(
                    o[:], g[:, 2, :], w[:, 2:3], o[:], op0=ALU.mult, op1=ALU.add
                )
                nc.sync.dma_start(out=out_d[b * 128 : (b + 1) * 128, :], in_=o[:])

    nc.compile()
    return nc


_PROGRAM_CACHE = {}


def _get_program(n_src, mq, c_feat):
    key = (n_src, mq, c_feat)
    if key not in _PROGRAM_CACHE:
        _PROGRAM_CACHE[key] = build_program(n_src, mq, c_feat)
    return _PROGRAM_CACHE[key]


def make_in_maps(xyz, new_xyz, feat):
    """Host-side shard + layout prep (slices/transposes only, no math)."""
    in_maps = []
    for core in range(N_CORES):
        b, h = divmod(core, 2)
        qs = new_xyz[b * M_QUERY + h * MQ : b * M_QUERY + (h + 1) * MQ]
        lhsT = np.zeros((5, MQ), np.float32)
        lhsT[0:3] = qs.T
        lhsT[3] = 1.0
        sT = np.zeros((5, N_SRC), np.float32)
        sT[0:3] = xyz[b * N_SRC : (b + 1) * N_SRC].T
        sT[4] = 1.0
        in_maps.append(
            {
                "lhsT": np.ascontiguousarray(lhsT),
                "sT": np.ascontiguousarray(sT),
                "feat": np.ascontiguousarray(
                    feat[b * N_SRC : (b + 1) * N_SRC], dtype=np.float32
                ),
            }
        )
    return in_maps


def kernel(xyz, new_xyz, feat, offset, new_offset, k):
    global LAST_RESULTS
    xyz = np.asarray(xyz, dtype=np.float32)
    new_xyz = np.asarray(new_xyz, dtype=np.float32)
    feat = np.asarray(feat, dtype=np.float32)
    assert int(np.asarray(k)) == KNN
    assert xyz.shape == (B_CLOUDS * N_SRC, 3), xyz.shape
    assert new_xyz.shape == (B_CLOUDS * M_QUERY, 3), new_xyz.shape
    assert feat.shape == (B_CLOUDS * N_SRC, C_FEAT), feat.shape

    nc = _get_program(N_SRC, MQ, C_FEAT)
    in_maps = make_in_maps(xyz, new_xyz, feat)

    res = run_bass_kernel_spmd(
        nc,
        in_maps,
        core_ids=list(range(N_CORES)),
        trace=bool(os.environ.get("BASS_TRACE")),
    )
    LAST_RESULTS = res

    out = np.empty((B_CLOUDS * M_QUERY, C_FEAT), np.float32)
    for core in range(N_CORES):
        b, h = divmod(core, 2)
        out[b * M_QUERY + h * MQ : b * M_QUERY + (h + 1) * MQ] = res.results[core][
            "out"
        ]
    return out
